# revision 6
# baseline (speedup 1.0000x reference)
"""Circular-relative-bias multi-head attention on 8 Trainium2 NeuronCores.

Sharding (Megatron MHA): 16 heads -> 2 heads per core. Each core computes
q/k/v projections for its 128 channels (2 heads x 64), full attention for
its heads over both batches, and a row-sharded output projection producing
a full-shape partial; the host sums the 8 partials and adds bo.

Layout strategy: the host pre-packs every input into the exact SBUF layout
the kernel wants, so all DMAs are linear:
  - xt      [1024, 4096]        x transposed (d-major)       f32
  - wq/wk/wv[128, 8, 128]       [k-part, d-tile, ch]         f32 (wq,bq pre-scaled 1/8)
  - wo      [128, 1024]         [ch, d]                      f32
  - bq/bk/bv[128, 1]            per-channel bias             f32
  - ebias   [128, 2, 28, 512]   exp(rel bias) tiles, by      bf16
                                [i, head, diag-class, j]

Attention works on transposed scores P^T [sk, sq] so softmax sums come free
from the attn@V matmul via a ones-column in the stationary operand:
  head0 lhsT = v_store[:, t, 0:65]  = [v0 | ones]      -> accumA rows 0-63 data, 64 sums
  head1 lhsT = v_store[:, t, 1:129] = [.. | ones | v1] -> accumB rows 64-127 data, 63 sums
This puts head1's outputs in partitions 64-127 directly (lane-aligned with
its slot in the [128, tok] attnout tile) at zero extra matmul cost.

exp(s + b) = exp(s) * exp(b): the circular bias enters as a precomputed
elementwise bf16 multiplier; [128, 512] score tiles along the same
(512*qb - 128*kt) diagonal share one of 28 classes per head.

No max-subtraction in softmax: scores ~ N(0,1) + 0.02-bias, |s| < ~7 over
4M samples, exp stays well inside f32 range.
"""

import math

import numpy as np
import ml_dtypes

B = 2
S = 2048
D = 1024
H = 16
HD = 64
PERIOD = 4096
NCORES = 8
CH = 128          # channels per core = 2 heads * 64
TOK = B * S       # 4096
DT = D // 128     # 8 k-tiles for the d contraction
TB = 256          # token block for projections
NTB = TOK // TB   # 16
SQ = 512          # sq block in attention
NQB = S // SQ     # 4 per batch
SK = 128          # sk tile
NKT = S // SK     # 16 per batch
NCLS = NQB * 4 + NKT - 4  # 28 diagonal classes: 4*qb - kt in [-15, 12]

_CACHE = {}


def _build_nc():
    import contextlib

    import concourse.tile as tile
    from concourse import bacc, mybir
    from concourse.masks import make_identity

    f32 = mybir.dt.float32
    f32r = mybir.dt.float32r
    bf16 = mybir.dt.bfloat16

    nc = bacc.Bacc("TRN2")
    xt = nc.dram_tensor("xt", [D, TOK], bf16, kind="ExternalInput")
    wq = nc.dram_tensor("wq", [128, DT, CH], bf16, kind="ExternalInput")
    wk = nc.dram_tensor("wk", [128, DT, CH], bf16, kind="ExternalInput")
    wv = nc.dram_tensor("wv", [128, DT, CH], bf16, kind="ExternalInput")
    wo = nc.dram_tensor("wo", [CH, D], bf16, kind="ExternalInput")
    bq = nc.dram_tensor("bq", [CH, 1], f32, kind="ExternalInput")
    bk = nc.dram_tensor("bk", [CH, 1], f32, kind="ExternalInput")
    bv = nc.dram_tensor("bv", [CH, 1], f32, kind="ExternalInput")
    ebias = nc.dram_tensor("ebias", [128, 2, NCLS, SQ], bf16, kind="ExternalInput")
    o_part = nc.dram_tensor("o_part", [TOK, D], f32, kind="ExternalOutput")

    with tile.TileContext(nc) as tc, contextlib.ExitStack() as ctx:
        singles = ctx.enter_context(tc.tile_pool(name="singles", bufs=1))
        xt_pool = ctx.enter_context(tc.tile_pool(name="xt", bufs=2))
        vt_pool = ctx.enter_context(tc.tile_pool(name="vt", bufs=2))
        ep_pool = ctx.enter_context(tc.tile_pool(name="ep", bufs=3))
        nrm_pool = ctx.enter_context(tc.tile_pool(name="nrm", bufs=2))
        ao_pool = ctx.enter_context(tc.tile_pool(name="ao", bufs=2))
        out_pool = ctx.enter_context(tc.tile_pool(name="out", bufs=2))
        mm_ps = ctx.enter_context(tc.tile_pool(name="mmps", bufs=4, space="PSUM"))
        acc_ps = ctx.enter_context(tc.tile_pool(name="accps", bufs=2, space="PSUM"))

        ident = singles.tile([128, 128], bf16)
        make_identity(nc, ident)

        w_sb = {}
        b_sb = {}
        for name, w_h, b_h in (("q", wq, bq), ("k", wk, bk), ("v", wv, bv)):
            w_sb[name] = singles.tile([128, DT, CH], bf16, tag=f"w{name}", name=f"w{name}_sb")
            nc.sync.dma_start(out=w_sb[name], in_=w_h[:, :, :])
            b_sb[name] = singles.tile([CH, 1], f32, tag=f"b{name}", name=f"b{name}_sb")
            nc.sync.dma_start(out=b_sb[name], in_=b_h[:, :])
        wo_sb = singles.tile([CH, D], bf16, tag="wo")
        nc.sync.dma_start(out=wo_sb, in_=wo[:, :])
        eb_sb = singles.tile([128, 2, NCLS, SQ], bf16, tag="eb")
        nc.sync.dma_start(out=eb_sb, in_=ebias[:, :, :, :])

        # q^T / k^T stores [ch, tok]; v_store [tok-part, tok-tile, 129]
        qT = singles.tile([CH, TOK], f32r, tag="qT")
        kT = singles.tile([CH, TOK], f32r, tag="kT")
        # v_store cols: [v0: 0..63 | ones: 64 | zeros: 65..95 | v1: 96..159]
        # head0 lhsT = [:, t, 0:65]   -> acc rows 0-63 data, 64 sums
        # head1 lhsT = [:, t, 32:160] -> acc row 32 sums, rows 64-127 data
        # (rows 0-31/33-63 of accB get junk from v0 cols — never read)
        v_store = singles.tile([128, TOK // 128, 160], bf16, tag="vst")
        nc.vector.memset(v_store[:, :, 64:65], 1.0)
        nc.vector.memset(v_store[:, :, 65:96], 0.0)

        xt_v = xt.rearrange("(dt k) t -> k dt t", k=128)

        # ---- projections ----
        for tb in range(NTB):
            ts = tb * TB
            xt_sb = xt_pool.tile([128, DT, TB], bf16, tag="xt")
            nc.sync.dma_start(out=xt_sb, in_=xt_v[:, :, ts : ts + TB])
            for name, store in (("q", qT), ("k", kT), ("v", None)):
                ps = mm_ps.tile([CH, TB], f32, tag="mm")
                for dt in range(DT):
                    nc.tensor.matmul(
                        ps,
                        w_sb[name][:, dt],
                        xt_sb[:, dt],
                        start=(dt == 0),
                        stop=(dt == DT - 1),
                    )
                if store is not None:
                    nc.vector.tensor_scalar_add(
                        out=store[:, ts : ts + TB], in0=ps, scalar1=b_sb[name]
                    )
                else:
                    vt_sb = vt_pool.tile([CH, TB], bf16, tag="vt")
                    nc.vector.tensor_scalar_add(
                        out=vt_sb, in0=ps, scalar1=b_sb["v"]
                    )
                    # transpose v^T -> v rows, split heads into v_store
                    for j in range(TB // 128):
                        t_idx = (ts + j * 128) // 128
                        vps = mm_ps.tile([128, 128], bf16, tag="mm")
                        nc.tensor.transpose(
                            vps, vt_sb[:, j * 128 : (j + 1) * 128], ident
                        )
                        # cols 0:64 -> head0 slot, cols 64:128 -> head1 slot
                        nc.vector.tensor_copy(
                            v_store[:, t_idx, 0:64], vps[:, 0:64]
                        )
                        nc.vector.tensor_copy(
                            v_store[:, t_idx, 96:160], vps[:, 64:128]
                        )

        # ---- attention + output projection, per batch ----
        for b in range(B):
            base = b * S
            attnout = ao_pool.tile([CH, S], bf16, tag="ao")
            for qb in range(NQB):
                q0 = base + qb * SQ
                accA = acc_ps.tile([128, SQ], f32, tag="acc")
                accB = acc_ps.tile([128, SQ], f32, tag="acc")
                for kt in range(NKT):
                    k0 = base + kt * SK
                    cls = 4 * qb - kt + (NKT - 1)
                    t_idx = k0 // 128
                    for hh, acc in ((0, accA), (1, accB)):
                        ps = mm_ps.tile([128, SQ], f32, tag="mm")
                        nc.tensor.matmul(
                            ps,
                            kT[hh * 64 : (hh + 1) * 64, k0 : k0 + SK],
                            qT[hh * 64 : (hh + 1) * 64, q0 : q0 + SQ],
                            start=True,
                            stop=True,
                        )
                        e_sb = ep_pool.tile([128, SQ], f32, tag="e")
                        nc.scalar.activation(
                            out=e_sb, in_=ps, func=mybir.ActivationFunctionType.Exp
                        )
                        p_sb = ep_pool.tile([128, SQ], bf16, tag="p")
                        nc.vector.tensor_mul(p_sb, e_sb, eb_sb[:, hh, cls])
                        lo, width = (0, 65) if hh == 0 else (32, 128)
                        nc.tensor.matmul(
                            acc[0:width, :],
                            v_store[:, t_idx, lo : lo + width],
                            p_sb,
                            start=(kt == 0),
                            stop=(kt == NKT - 1),
                        )
                # normalize: head0 sums at accA row 64, head1 sums at accB row 63
                for hh, acc, srow in ((0, accA, 64), (1, accB, 32)):
                    r = nrm_pool.tile([1, SQ], f32, tag="r")
                    nc.vector.reciprocal(r, acc[srow : srow + 1, :])
                    rb = nrm_pool.tile([64, SQ], f32, tag="rb")
                    nc.gpsimd.partition_broadcast(rb, r)
                    dlo = 0 if hh == 0 else 64
                    nc.vector.tensor_mul(
                        attnout[dlo : dlo + 64, qb * SQ : (qb + 1) * SQ],
                        acc[dlo : dlo + 64, :],
                        rb,
                    )
            # ---- output projection for this batch ----
            for ts in range(S // 128):
                o_sb = out_pool.tile([128, D], f32, tag="o")
                for half in range(2):
                    ps = mm_ps.tile([128, 512], f32, tag="mm")
                    nc.tensor.matmul(
                        ps,
                        attnout[:, ts * 128 : (ts + 1) * 128],
                        wo_sb[:, half * 512 : (half + 1) * 512],
                        start=True,
                        stop=True,
                    )
                    nc.scalar.copy(o_sb[:, half * 512 : (half + 1) * 512], ps)
                nc.sync.dma_start(
                    out=o_part[base + ts * 128 : base + (ts + 1) * 128, :], in_=o_sb
                )
    nc.compile()
    return nc


def _prep_inputs(x, wq, bq, wk, bk, wv, bv, wo, bo, rel_bias):
    """Host-side pack into per-core in_maps (all linear-DMA layouts)."""
    x = np.asarray(x, dtype=np.float32)
    rel_bias = np.asarray(rel_bias, dtype=np.float32)
    scale = 1.0 / math.sqrt(HD)

    xt = np.ascontiguousarray(x.reshape(TOK, D).T).astype(ml_dtypes.bfloat16)  # [D, TOK]

    # exp-bias tiles: ebt[i, hh, cls, j] = exp(rel_bias[(c0 - i + j) % PERIOD, h])
    ii = np.arange(128)[:, None]
    jj = np.arange(SQ)[None, :]
    cls_idx = np.empty((NCLS, 128, SQ), dtype=np.int64)
    for cls in range(NCLS):
        c0 = 128 * (cls - (NKT - 1))
        cls_idx[cls] = (c0 - ii + jj) % PERIOD

    in_maps = []
    for c in range(NCORES):
        sl = slice(c * CH, (c + 1) * CH)
        wq_c = (np.asarray(wq, np.float32)[:, sl] * scale).reshape(DT, 128, CH)
        wk_c = np.asarray(wk, np.float32)[:, sl].reshape(DT, 128, CH)
        wv_c = np.asarray(wv, np.float32)[:, sl].reshape(DT, 128, CH)
        eb = np.empty((128, 2, NCLS, SQ), dtype=ml_dtypes.bfloat16)
        for hh in range(2):
            h = 2 * c + hh
            # [NCLS, 128, SQ] -> [128, NCLS, SQ]
            eb[:, hh] = np.exp(rel_bias[cls_idx, h]).transpose(1, 0, 2)
        in_maps.append(
            {
                "xt": xt,
                "wq": np.ascontiguousarray(wq_c.transpose(1, 0, 2)).astype(ml_dtypes.bfloat16),
                "wk": np.ascontiguousarray(wk_c.transpose(1, 0, 2)).astype(ml_dtypes.bfloat16),
                "wv": np.ascontiguousarray(wv_c.transpose(1, 0, 2)).astype(ml_dtypes.bfloat16),
                "wo": np.ascontiguousarray(np.asarray(wo, np.float32)[sl, :]).astype(ml_dtypes.bfloat16),
                "bq": (np.asarray(bq, np.float32)[sl] * scale).reshape(CH, 1),
                "bk": np.asarray(bk, np.float32)[sl].reshape(CH, 1),
                "bv": np.asarray(bv, np.float32)[sl].reshape(CH, 1),
                "ebias": eb,
            }
        )
    return in_maps


def kernel(x, wq, bq, wk, bk, wv, bv, wo, bo, rel_bias, _trace=False):
    from concourse import bass_utils

    if "nc" not in _CACHE:
        _CACHE["nc"] = _build_nc()
    nc = _CACHE["nc"]

    in_maps = _prep_inputs(x, wq, bq, wk, bk, wv, bv, wo, bo, rel_bias)
    res = bass_utils.run_bass_kernel_spmd(
        nc, in_maps, core_ids=list(range(NCORES)), trace=_trace
    )
    _CACHE["last_result"] = res

    acc = np.zeros((TOK, D), dtype=np.float64)
    for r in res.results:
        acc += r["o_part"].astype(np.float64)
    acc += np.asarray(bo, np.float64)[None, :]
    return acc.reshape(B, S, D).astype(np.float32)


# revision 7
# speedup vs baseline: 1.3295x; 1.3295x over previous
"""Circular-relative-bias multi-head attention on 8 Trainium2 NeuronCores.

Sharding (Megatron MHA): 16 heads -> 2 heads per core. Each core computes
q/k/v projections for its 128 channels (2 heads x 64), full attention for
its heads over both batches, and a row-sharded output projection producing
a full-shape partial; the host sums the 8 partials and adds bo.

Layout strategy: the host pre-packs every input into the exact SBUF layout
the kernel wants, so all DMAs are linear:
  - xt      [1024, 4096]        x transposed (d-major)       f32
  - wq/wk/wv[128, 8, 128]       [k-part, d-tile, ch]         f32 (wq,bq pre-scaled 1/8)
  - wo      [128, 1024]         [ch, d]                      f32
  - bq/bk/bv[128, 1]            per-channel bias             f32
  - ebias   [128, 2, 28, 512]   exp(rel bias) tiles, by      bf16
                                [i, head, diag-class, j]

Attention works on transposed scores P^T [sk, sq] so softmax sums come free
from the attn@V matmul via a ones-column in the stationary operand:
  head0 lhsT = v_store[:, t, 0:65]  = [v0 | ones]      -> accumA rows 0-63 data, 64 sums
  head1 lhsT = v_store[:, t, 1:129] = [.. | ones | v1] -> accumB rows 64-127 data, 63 sums
This puts head1's outputs in partitions 64-127 directly (lane-aligned with
its slot in the [128, tok] attnout tile) at zero extra matmul cost.

exp(s + b) = exp(s) * exp(b): the circular bias enters as a precomputed
elementwise bf16 multiplier; [128, 512] score tiles along the same
(512*qb - 128*kt) diagonal share one of 28 classes per head.

No max-subtraction in softmax: scores ~ N(0,1) + 0.02-bias, |s| < ~7 over
4M samples, exp stays well inside f32 range.
"""

import math

import numpy as np
import ml_dtypes

B = 2
S = 2048
D = 1024
H = 16
HD = 64
PERIOD = 4096
NCORES = 8
CH = 128          # channels per core = 2 heads * 64
TOK = B * S       # 4096
DT = D // 128     # 8 k-tiles for the d contraction
TB = 256          # token block for projections
NTB = TOK // TB   # 16
SQ = 512          # sq block in attention
NQB = S // SQ     # 4 per batch
SK = 128          # sk tile
NKT = S // SK     # 16 per batch
NCLS = NQB * 4 + NKT - 4  # 28 diagonal classes: 4*qb - kt in [-15, 12]

_CACHE = {}


def _build_nc():
    import contextlib

    import concourse.tile as tile
    from concourse import bacc, mybir
    from concourse.masks import make_identity

    f32 = mybir.dt.float32
    f32r = mybir.dt.float32r
    bf16 = mybir.dt.bfloat16

    nc = bacc.Bacc("TRN2")
    xt = nc.dram_tensor("xt", [D, TOK], bf16, kind="ExternalInput")
    wq = nc.dram_tensor("wq", [128, DT, CH], bf16, kind="ExternalInput")
    wk = nc.dram_tensor("wk", [128, DT, CH], bf16, kind="ExternalInput")
    wv = nc.dram_tensor("wv", [128, DT, CH], bf16, kind="ExternalInput")
    wo = nc.dram_tensor("wo", [CH, D], bf16, kind="ExternalInput")
    bq = nc.dram_tensor("bq", [CH, 1], f32, kind="ExternalInput")
    bk = nc.dram_tensor("bk", [CH, 1], f32, kind="ExternalInput")
    bv = nc.dram_tensor("bv", [CH, 1], f32, kind="ExternalInput")
    ebias = nc.dram_tensor("ebias", [128, 2, NCLS, SQ], bf16, kind="ExternalInput")
    o_part = nc.dram_tensor("o_part", [TOK, D], f32, kind="ExternalOutput")

    with tile.TileContext(nc) as tc, contextlib.ExitStack() as ctx:
        singles = ctx.enter_context(tc.tile_pool(name="singles", bufs=1))
        xt_pool = ctx.enter_context(tc.tile_pool(name="xt", bufs=2))
        vt_pool = ctx.enter_context(tc.tile_pool(name="vt", bufs=2))
        ep_pool = ctx.enter_context(tc.tile_pool(name="ep", bufs=4))
        nrm_pool = ctx.enter_context(tc.tile_pool(name="nrm", bufs=2))
        ao_pool = ctx.enter_context(tc.tile_pool(name="ao", bufs=2))
        out_pool = ctx.enter_context(tc.tile_pool(name="out", bufs=2))
        mm_ps = ctx.enter_context(tc.tile_pool(name="mmps", bufs=3, space="PSUM"))
        acc_ps = ctx.enter_context(tc.tile_pool(name="accps", bufs=2, space="PSUM"))

        ident = singles.tile([128, 128], bf16)
        make_identity(nc, ident)

        w_sb = {}
        b_sb = {}
        for name, w_h, b_h in (("q", wq, bq), ("k", wk, bk), ("v", wv, bv)):
            w_sb[name] = singles.tile([128, DT, CH], bf16, tag=f"w{name}", name=f"w{name}_sb")
            nc.sync.dma_start(out=w_sb[name], in_=w_h[:, :, :])
            b_sb[name] = singles.tile([CH, 1], f32, tag=f"b{name}", name=f"b{name}_sb")
            nc.sync.dma_start(out=b_sb[name], in_=b_h[:, :])
        wo_sb = singles.tile([CH, D], bf16, tag="wo")
        nc.sync.dma_start(out=wo_sb, in_=wo[:, :])
        eb_sb = singles.tile([128, 2, NCLS, SQ], bf16, tag="eb")
        nc.sync.dma_start(out=eb_sb, in_=ebias[:, :, :, :])

        # q^T / k^T stores [ch, tok]; v_store [tok-part, tok-tile, 129]
        qT = singles.tile([CH, TOK], f32r, tag="qT")
        kT = singles.tile([CH, TOK], f32r, tag="kT")
        # v_store cols: [v0: 0..63 | ones: 64 | zeros: 65..95 | v1: 96..159]
        # head0 lhsT = [:, t, 0:65]   -> acc rows 0-63 data, 64 sums
        # head1 lhsT = [:, t, 32:160] -> acc row 32 sums, rows 64-127 data
        # (rows 0-31/33-63 of accB get junk from v0 cols — never read)
        v_store = singles.tile([128, TOK // 128, 160], bf16, tag="vst")
        nc.vector.memset(v_store[:, :, 64:65], 1.0)
        nc.vector.memset(v_store[:, :, 65:96], 0.0)

        xt_v = xt.rearrange("(dt k) t -> k dt t", k=128)

        # ---- projections ----
        for tb in range(NTB):
            ts = tb * TB
            xt_sb = xt_pool.tile([128, DT, TB], bf16, tag="xt")
            nc.sync.dma_start(out=xt_sb, in_=xt_v[:, :, ts : ts + TB])
            for name, store in (("q", qT), ("k", kT), ("v", None)):
                ps = mm_ps.tile([CH, TB], f32, tag="mm")
                for dt in range(DT):
                    nc.tensor.matmul(
                        ps,
                        w_sb[name][:, dt],
                        xt_sb[:, dt],
                        start=(dt == 0),
                        stop=(dt == DT - 1),
                    )
                if store is not None:
                    nc.vector.tensor_scalar_add(
                        out=store[:, ts : ts + TB], in0=ps, scalar1=b_sb[name]
                    )
                else:
                    vt_sb = vt_pool.tile([CH, TB], bf16, tag="vt")
                    nc.vector.tensor_scalar_add(
                        out=vt_sb, in0=ps, scalar1=b_sb["v"]
                    )
                    # transpose v^T -> v rows, split heads into v_store
                    for j in range(TB // 128):
                        t_idx = (ts + j * 128) // 128
                        vps = mm_ps.tile([128, 128], bf16, tag="mm")
                        nc.tensor.transpose(
                            vps, vt_sb[:, j * 128 : (j + 1) * 128], ident
                        )
                        # cols 0:64 -> head0 slot, cols 64:128 -> head1 slot
                        nc.vector.tensor_copy(
                            v_store[:, t_idx, 0:64], vps[:, 0:64]
                        )
                        nc.vector.tensor_copy(
                            v_store[:, t_idx, 96:160], vps[:, 64:128]
                        )

        # ---- attention + output projection, per batch ----
        for b in range(B):
            base = b * S
            attnout = ao_pool.tile([CH, S], bf16, tag="ao")
            for qb in range(NQB):
                q0 = base + qb * SQ
                accA = acc_ps.tile([128, SQ], f32, tag="acc")
                accB = acc_ps.tile([128, SQ], f32, tag="acc")
                for kt in range(NKT):
                    k0 = base + kt * SK
                    cls = 4 * qb - kt + (NKT - 1)
                    t_idx = k0 // 128
                    # both heads' score tiles in one 2-bank psum tile; the two
                    # QK matmuls are adjacent and use disjoint row groups
                    # (base partitions 0 / 64) so they overlap in the array
                    ps = mm_ps.tile([128, 2, SQ], f32, tag="mm")
                    for hh in (0, 1):
                        nc.tensor.matmul(
                            ps[:, hh, :],
                            kT[hh * 64 : (hh + 1) * 64, k0 : k0 + SK],
                            qT[hh * 64 : (hh + 1) * 64, q0 : q0 + SQ],
                            start=True,
                            stop=True,
                        )
                    e_sb = ep_pool.tile([128, 2, SQ], bf16, tag="e")
                    nc.scalar.activation(
                        out=e_sb, in_=ps, func=mybir.ActivationFunctionType.Exp
                    )
                    p_sb = ep_pool.tile([128, 2, SQ], bf16, tag="p")
                    nc.vector.tensor_mul(p_sb, e_sb, eb_sb[:, :, cls, :])
                    for hh, acc in ((0, accA), (1, accB)):
                        lo, width = (0, 65) if hh == 0 else (32, 128)
                        nc.tensor.matmul(
                            acc[0:width, :],
                            v_store[:, t_idx, lo : lo + width],
                            p_sb[:, hh, :],
                            start=(kt == 0),
                            stop=(kt == NKT - 1),
                        )
                # normalize: head0 sums at accA row 64, head1 sums at accB row 63
                for hh, acc, srow in ((0, accA, 64), (1, accB, 32)):
                    r = nrm_pool.tile([1, SQ], f32, tag="r")
                    nc.vector.reciprocal(r, acc[srow : srow + 1, :])
                    rb = nrm_pool.tile([64, SQ], f32, tag="rb")
                    nc.gpsimd.partition_broadcast(rb, r)
                    dlo = 0 if hh == 0 else 64
                    nc.vector.tensor_mul(
                        attnout[dlo : dlo + 64, qb * SQ : (qb + 1) * SQ],
                        acc[dlo : dlo + 64, :],
                        rb,
                    )
            # ---- output projection for this batch ----
            for ts in range(S // 128):
                o_sb = out_pool.tile([128, D], f32, tag="o")
                for half in range(2):
                    ps = mm_ps.tile([128, 512], f32, tag="mm")
                    nc.tensor.matmul(
                        ps,
                        attnout[:, ts * 128 : (ts + 1) * 128],
                        wo_sb[:, half * 512 : (half + 1) * 512],
                        start=True,
                        stop=True,
                    )
                    nc.scalar.copy(o_sb[:, half * 512 : (half + 1) * 512], ps)
                nc.sync.dma_start(
                    out=o_part[base + ts * 128 : base + (ts + 1) * 128, :], in_=o_sb
                )
    nc.compile()
    return nc


def _prep_inputs(x, wq, bq, wk, bk, wv, bv, wo, bo, rel_bias):
    """Host-side pack into per-core in_maps (all linear-DMA layouts)."""
    x = np.asarray(x, dtype=np.float32)
    rel_bias = np.asarray(rel_bias, dtype=np.float32)
    scale = 1.0 / math.sqrt(HD)

    xt = np.ascontiguousarray(x.reshape(TOK, D).T).astype(ml_dtypes.bfloat16)  # [D, TOK]

    # exp-bias tiles: ebt[i, hh, cls, j] = exp(rel_bias[(c0 - i + j) % PERIOD, h])
    ii = np.arange(128)[:, None]
    jj = np.arange(SQ)[None, :]
    cls_idx = np.empty((NCLS, 128, SQ), dtype=np.int64)
    for cls in range(NCLS):
        c0 = 128 * (cls - (NKT - 1))
        cls_idx[cls] = (c0 - ii + jj) % PERIOD

    in_maps = []
    for c in range(NCORES):
        sl = slice(c * CH, (c + 1) * CH)
        wq_c = (np.asarray(wq, np.float32)[:, sl] * scale).reshape(DT, 128, CH)
        wk_c = np.asarray(wk, np.float32)[:, sl].reshape(DT, 128, CH)
        wv_c = np.asarray(wv, np.float32)[:, sl].reshape(DT, 128, CH)
        eb = np.empty((128, 2, NCLS, SQ), dtype=ml_dtypes.bfloat16)
        for hh in range(2):
            h = 2 * c + hh
            # [NCLS, 128, SQ] -> [128, NCLS, SQ]
            eb[:, hh] = np.exp(rel_bias[cls_idx, h]).transpose(1, 0, 2)
        in_maps.append(
            {
                "xt": xt,
                "wq": np.ascontiguousarray(wq_c.transpose(1, 0, 2)).astype(ml_dtypes.bfloat16),
                "wk": np.ascontiguousarray(wk_c.transpose(1, 0, 2)).astype(ml_dtypes.bfloat16),
                "wv": np.ascontiguousarray(wv_c.transpose(1, 0, 2)).astype(ml_dtypes.bfloat16),
                "wo": np.ascontiguousarray(np.asarray(wo, np.float32)[sl, :]).astype(ml_dtypes.bfloat16),
                "bq": (np.asarray(bq, np.float32)[sl] * scale).reshape(CH, 1),
                "bk": np.asarray(bk, np.float32)[sl].reshape(CH, 1),
                "bv": np.asarray(bv, np.float32)[sl].reshape(CH, 1),
                "ebias": eb,
            }
        )
    return in_maps


def kernel(x, wq, bq, wk, bk, wv, bv, wo, bo, rel_bias, _trace=False):
    from concourse import bass_utils

    if "nc" not in _CACHE:
        _CACHE["nc"] = _build_nc()
    nc = _CACHE["nc"]

    in_maps = _prep_inputs(x, wq, bq, wk, bk, wv, bv, wo, bo, rel_bias)
    res = bass_utils.run_bass_kernel_spmd(
        nc, in_maps, core_ids=list(range(NCORES)), trace=_trace
    )
    _CACHE["last_result"] = res

    acc = np.zeros((TOK, D), dtype=np.float64)
    for r in res.results:
        acc += r["o_part"].astype(np.float64)
    acc += np.asarray(bo, np.float64)[None, :]
    return acc.reshape(B, S, D).astype(np.float32)


# revision 8
# speedup vs baseline: 1.3585x; 1.0218x over previous
"""Circular-relative-bias multi-head attention on 8 Trainium2 NeuronCores.

Sharding (Megatron MHA): 16 heads -> 2 heads per core. Each core computes
q/k/v projections for its 128 channels (2 heads x 64), full attention for
its heads over both batches, and a row-sharded output projection producing
a full-shape partial; the host sums the 8 partials and adds bo.

Layout strategy: the host pre-packs every input into the exact SBUF layout
the kernel wants, so all DMAs are linear:
  - xt      [1024, 4096]        x transposed (d-major)       f32
  - wq/wk/wv[128, 8, 128]       [k-part, d-tile, ch]         f32 (wq,bq pre-scaled 1/8)
  - wo      [128, 1024]         [ch, d]                      f32
  - bq/bk/bv[128, 1]            per-channel bias             f32
  - ebias   [128, 2, 28, 512]   exp(rel bias) tiles, by      bf16
                                [i, head, diag-class, j]

Attention works on transposed scores P^T [sk, sq] so softmax sums come free
from the attn@V matmul via a ones-column in the stationary operand:
  head0 lhsT = v_store[:, t, 0:65]  = [v0 | ones]      -> accumA rows 0-63 data, 64 sums
  head1 lhsT = v_store[:, t, 1:129] = [.. | ones | v1] -> accumB rows 64-127 data, 63 sums
This puts head1's outputs in partitions 64-127 directly (lane-aligned with
its slot in the [128, tok] attnout tile) at zero extra matmul cost.

exp(s + b) = exp(s) * exp(b): the circular bias enters as a precomputed
elementwise bf16 multiplier; [128, 512] score tiles along the same
(512*qb - 128*kt) diagonal share one of 28 classes per head.

No max-subtraction in softmax: scores ~ N(0,1) + 0.02-bias, |s| < ~7 over
4M samples, exp stays well inside f32 range.
"""

import math

import numpy as np
import ml_dtypes

B = 2
S = 2048
D = 1024
H = 16
HD = 64
PERIOD = 4096
NCORES = 8
CH = 128          # channels per core = 2 heads * 64
TOK = B * S       # 4096
DT = D // 128     # 8 k-tiles for the d contraction
TB = 256          # token block for projections
NTB = TOK // TB   # 16
SQ = 512          # sq block in attention
NQB = S // SQ     # 4 per batch
SK = 128          # sk tile
NKT = S // SK     # 16 per batch
NCLS = NQB * 4 + NKT - 4  # 28 diagonal classes: 4*qb - kt in [-15, 12]

_CACHE = {}


def _build_nc():
    import contextlib

    import concourse.tile as tile
    from concourse import bacc, mybir
    from concourse.masks import make_identity

    f32 = mybir.dt.float32
    f32r = mybir.dt.float32r
    bf16 = mybir.dt.bfloat16

    nc = bacc.Bacc("TRN2")
    xt = nc.dram_tensor("xt", [D, TOK], bf16, kind="ExternalInput")
    wq = nc.dram_tensor("wq", [128, DT, CH], bf16, kind="ExternalInput")
    wk = nc.dram_tensor("wk", [128, DT, CH], bf16, kind="ExternalInput")
    wv = nc.dram_tensor("wv", [128, DT, CH], bf16, kind="ExternalInput")
    wo = nc.dram_tensor("wo", [CH, D], bf16, kind="ExternalInput")
    bq = nc.dram_tensor("bq", [CH, 1], f32, kind="ExternalInput")
    bk = nc.dram_tensor("bk", [CH, 1], f32, kind="ExternalInput")
    bv = nc.dram_tensor("bv", [CH, 1], f32, kind="ExternalInput")
    ebias = nc.dram_tensor("ebias", [128, 2, NCLS, SQ], bf16, kind="ExternalInput")
    o_part = nc.dram_tensor("o_part", [TOK, D], f32, kind="ExternalOutput")

    with tile.TileContext(nc) as tc, contextlib.ExitStack() as ctx:
        singles = ctx.enter_context(tc.tile_pool(name="singles", bufs=1))
        xt_pool = ctx.enter_context(tc.tile_pool(name="xt", bufs=2))
        vt_pool = ctx.enter_context(tc.tile_pool(name="vt", bufs=2))
        ep_pool = ctx.enter_context(tc.tile_pool(name="ep", bufs=4))
        nrm_pool = ctx.enter_context(tc.tile_pool(name="nrm", bufs=2))
        ao_pool = ctx.enter_context(tc.tile_pool(name="ao", bufs=2))
        out_pool = ctx.enter_context(tc.tile_pool(name="out", bufs=2))
        mm_ps = ctx.enter_context(tc.tile_pool(name="mmps", bufs=3, space="PSUM"))
        acc_ps = ctx.enter_context(tc.tile_pool(name="accps", bufs=2, space="PSUM"))

        ident = singles.tile([128, 128], bf16)
        make_identity(nc, ident)

        w_sb = {}
        b_sb = {}
        for name, w_h, b_h in (("q", wq, bq), ("k", wk, bk), ("v", wv, bv)):
            w_sb[name] = singles.tile([128, DT, CH], bf16, tag=f"w{name}", name=f"w{name}_sb")
            nc.sync.dma_start(out=w_sb[name], in_=w_h[:, :, :])
            b_sb[name] = singles.tile([CH, 1], f32, tag=f"b{name}", name=f"b{name}_sb")
            nc.sync.dma_start(out=b_sb[name], in_=b_h[:, :])
        wo_sb = singles.tile([CH, D], bf16, tag="wo")
        nc.sync.dma_start(out=wo_sb, in_=wo[:, :])
        eb_sb = singles.tile([128, 2, NCLS, SQ], bf16, tag="eb")
        nc.sync.dma_start(out=eb_sb, in_=ebias[:, :, :, :])

        # q^T / k^T stores [ch, tok]; v_store [tok-part, tok-tile, 129]
        qT = singles.tile([CH, TOK], bf16, tag="qT")
        kT = singles.tile([CH, TOK], bf16, tag="kT")
        # v_store cols: [v0: 0..63 | ones: 64 | zeros: 65..95 | v1: 96..159]
        # head0 lhsT = [:, t, 0:65]   -> acc rows 0-63 data, 64 sums
        # head1 lhsT = [:, t, 32:160] -> acc row 32 sums, rows 64-127 data
        # (rows 0-31/33-63 of accB get junk from v0 cols — never read)
        v_store = singles.tile([128, TOK // 128, 160], bf16, tag="vst")
        nc.vector.memset(v_store[:, :, 64:65], 1.0)
        nc.vector.memset(v_store[:, :, 65:96], 0.0)

        xt_v = xt.rearrange("(dt k) t -> k dt t", k=128)

        # ---- projections ----
        for tb in range(NTB):
            ts = tb * TB
            xt_sb = xt_pool.tile([128, DT, TB], bf16, tag="xt")
            nc.sync.dma_start(out=xt_sb, in_=xt_v[:, :, ts : ts + TB])
            for name, store in (("q", qT), ("k", kT), ("v", None)):
                ps = mm_ps.tile([CH, TB], f32, tag="mm")
                for dt in range(DT):
                    nc.tensor.matmul(
                        ps,
                        w_sb[name][:, dt],
                        xt_sb[:, dt],
                        start=(dt == 0),
                        stop=(dt == DT - 1),
                    )
                if store is not None:
                    nc.vector.tensor_scalar_add(
                        out=store[:, ts : ts + TB], in0=ps, scalar1=b_sb[name]
                    )
                else:
                    vt_sb = vt_pool.tile([CH, TB], bf16, tag="vt")
                    nc.vector.tensor_scalar_add(
                        out=vt_sb, in0=ps, scalar1=b_sb["v"]
                    )
                    # transpose v^T -> v rows, split heads into v_store
                    for j in range(TB // 128):
                        t_idx = (ts + j * 128) // 128
                        vps = mm_ps.tile([128, 128], bf16, tag="mm")
                        nc.tensor.transpose(
                            vps, vt_sb[:, j * 128 : (j + 1) * 128], ident
                        )
                        # cols 0:64 -> head0 slot, cols 64:128 -> head1 slot
                        nc.vector.tensor_copy(
                            v_store[:, t_idx, 0:64], vps[:, 0:64]
                        )
                        nc.vector.tensor_copy(
                            v_store[:, t_idx, 96:160], vps[:, 64:128]
                        )

        # ---- attention + output projection, per batch ----
        for b in range(B):
            base = b * S
            attnout = ao_pool.tile([CH, S], bf16, tag="ao")
            for qb in range(NQB):
                q0 = base + qb * SQ
                accA = acc_ps.tile([128, SQ], f32, tag="acc")
                accB = acc_ps.tile([128, SQ], f32, tag="acc")
                for kt in range(NKT):
                    k0 = base + kt * SK
                    cls = 4 * qb - kt + (NKT - 1)
                    t_idx = k0 // 128
                    # both heads' score tiles in one 2-bank psum tile; the two
                    # QK matmuls are adjacent and use disjoint row groups
                    # (base partitions 0 / 64) so they overlap in the array
                    ps = mm_ps.tile([128, 2, SQ], f32, tag="mm")
                    for hh in (0, 1):
                        nc.tensor.matmul(
                            ps[:, hh, :],
                            kT[hh * 64 : (hh + 1) * 64, k0 : k0 + SK],
                            qT[hh * 64 : (hh + 1) * 64, q0 : q0 + SQ],
                            start=True,
                            stop=True,
                        )
                    e_sb = ep_pool.tile([128, 2, SQ], bf16, tag="e")
                    nc.scalar.activation(
                        out=e_sb, in_=ps, func=mybir.ActivationFunctionType.Exp
                    )
                    p_sb = ep_pool.tile([128, 2, SQ], bf16, tag="p")
                    nc.vector.tensor_mul(p_sb, e_sb, eb_sb[:, :, cls, :])
                    for hh, acc in ((0, accA), (1, accB)):
                        lo, width = (0, 65) if hh == 0 else (32, 128)
                        nc.tensor.matmul(
                            acc[0:width, :],
                            v_store[:, t_idx, lo : lo + width],
                            p_sb[:, hh, :],
                            start=(kt == 0),
                            stop=(kt == NKT - 1),
                        )
                # normalize: head0 sums at accA row 64, head1 sums at accB row 63
                for hh, acc, srow in ((0, accA, 64), (1, accB, 32)):
                    r = nrm_pool.tile([1, SQ], f32, tag="r")
                    nc.vector.reciprocal(r, acc[srow : srow + 1, :])
                    rb = nrm_pool.tile([64, SQ], f32, tag="rb")
                    nc.gpsimd.partition_broadcast(rb, r)
                    dlo = 0 if hh == 0 else 64
                    nc.vector.tensor_mul(
                        attnout[dlo : dlo + 64, qb * SQ : (qb + 1) * SQ],
                        acc[dlo : dlo + 64, :],
                        rb,
                    )
            # ---- output projection for this batch ----
            for ts in range(S // 128):
                o_sb = out_pool.tile([128, D], f32, tag="o")
                for half in range(2):
                    ps = mm_ps.tile([128, 512], f32, tag="mm")
                    nc.tensor.matmul(
                        ps,
                        attnout[:, ts * 128 : (ts + 1) * 128],
                        wo_sb[:, half * 512 : (half + 1) * 512],
                        start=True,
                        stop=True,
                    )
                    nc.vector.tensor_copy(o_sb[:, half * 512 : (half + 1) * 512], ps)
                nc.sync.dma_start(
                    out=o_part[base + ts * 128 : base + (ts + 1) * 128, :], in_=o_sb
                )
    nc.compile()
    return nc


def _prep_inputs(x, wq, bq, wk, bk, wv, bv, wo, bo, rel_bias):
    """Host-side pack into per-core in_maps (all linear-DMA layouts)."""
    x = np.asarray(x, dtype=np.float32)
    rel_bias = np.asarray(rel_bias, dtype=np.float32)
    scale = 1.0 / math.sqrt(HD)

    xt = np.ascontiguousarray(x.reshape(TOK, D).T).astype(ml_dtypes.bfloat16)  # [D, TOK]

    # exp-bias tiles: ebt[i, hh, cls, j] = exp(rel_bias[(c0 - i + j) % PERIOD, h])
    ii = np.arange(128)[:, None]
    jj = np.arange(SQ)[None, :]
    cls_idx = np.empty((NCLS, 128, SQ), dtype=np.int64)
    for cls in range(NCLS):
        c0 = 128 * (cls - (NKT - 1))
        cls_idx[cls] = (c0 - ii + jj) % PERIOD

    in_maps = []
    for c in range(NCORES):
        sl = slice(c * CH, (c + 1) * CH)
        wq_c = (np.asarray(wq, np.float32)[:, sl] * scale).reshape(DT, 128, CH)
        wk_c = np.asarray(wk, np.float32)[:, sl].reshape(DT, 128, CH)
        wv_c = np.asarray(wv, np.float32)[:, sl].reshape(DT, 128, CH)
        eb = np.empty((128, 2, NCLS, SQ), dtype=ml_dtypes.bfloat16)
        for hh in range(2):
            h = 2 * c + hh
            # [NCLS, 128, SQ] -> [128, NCLS, SQ]
            eb[:, hh] = np.exp(rel_bias[cls_idx, h]).transpose(1, 0, 2)
        in_maps.append(
            {
                "xt": xt,
                "wq": np.ascontiguousarray(wq_c.transpose(1, 0, 2)).astype(ml_dtypes.bfloat16),
                "wk": np.ascontiguousarray(wk_c.transpose(1, 0, 2)).astype(ml_dtypes.bfloat16),
                "wv": np.ascontiguousarray(wv_c.transpose(1, 0, 2)).astype(ml_dtypes.bfloat16),
                "wo": np.ascontiguousarray(np.asarray(wo, np.float32)[sl, :]).astype(ml_dtypes.bfloat16),
                "bq": (np.asarray(bq, np.float32)[sl] * scale).reshape(CH, 1),
                "bk": np.asarray(bk, np.float32)[sl].reshape(CH, 1),
                "bv": np.asarray(bv, np.float32)[sl].reshape(CH, 1),
                "ebias": eb,
            }
        )
    return in_maps


def kernel(x, wq, bq, wk, bk, wv, bv, wo, bo, rel_bias, _trace=False):
    from concourse import bass_utils

    if "nc" not in _CACHE:
        _CACHE["nc"] = _build_nc()
    nc = _CACHE["nc"]

    in_maps = _prep_inputs(x, wq, bq, wk, bk, wv, bv, wo, bo, rel_bias)
    res = bass_utils.run_bass_kernel_spmd(
        nc, in_maps, core_ids=list(range(NCORES)), trace=_trace
    )
    _CACHE["last_result"] = res

    acc = np.zeros((TOK, D), dtype=np.float64)
    for r in res.results:
        acc += r["o_part"].astype(np.float64)
    acc += np.asarray(bo, np.float64)[None, :]
    return acc.reshape(B, S, D).astype(np.float32)
